# revision 8
# baseline (speedup 1.0000x reference)
"""Trainium2 Bass kernel for nn_NSMCell (GNN message passing).

Strategy
--------
The reference output is only [N]: a per-graph blend of two segment softmaxes
over per-node scalars.  Both scalars are of the form

    s_i = sum_d w_d * elu( M_g[d, :] @ x_i )

where for "node items" M_g = (sim[g] . W_node_props) * instr[g] and x = node
attr, and for "edge items" M_g = W_edge * instr[g] and x = edge attr.  The
per-graph matrices are built on the host (they are tiny); the device streams
all item columns through the PE + exp/min elu + a weighted partition reduce.
Graphs (16 per core) are sharded across the 8 cores; every graph's node and
edge items are padded to fixed per-graph segment sizes so a single NEFF
serves all cores.  The edge-message scatter (index_add) collapses to a
host-side bincount of per-edge scalars, and the segment softmax + blend run
on the host over [N] values (negligible work).

Device layout per 512-item tile (d on partitions, 2 chunks of 128 side by
side in one 2-bank PSUM tile; all A matrices pre-scaled x16):
  y[d, e]   = A_seg[k, d]^T @ xT[k, e]   edge tiles: 2 fp8e4 DoubleRow
                                         matmuls (K=256 virtualized);
                                         node tiles: 4 bf16 matmuls (node
                                         y has ~4x the variance - fp8
                                         there costs ~4% logit error)
  E         = exp(y/16)                  ScalarE, PSUM -> SBUF bf16
  EL        = min(E, relu(y)/16 + 1)     one fused custom VectorE op
  s-rows   += (w (x) delta_c)^T @ EL     2 bf16 matmuls into separate PSUM
                                         col-groups (concurrent via 32-col
                                         array tiling); kc halves land in
                                         different row halves, host adds
The s accumulator is a 2-bank PSUM tile ([128, 1024]); tile t's 512 item
sums land at rows (t%64, 64+t%64), cols (t//64)*512.  Drained once at the
end; the host adds the kc half-rows and subtracts sum(w) to undo the +1.

The elementwise stage is the wall: DVE reads y from PSUM at 1 elem/lane/cyc
(custom DVE ops have no 2x mode and the PSUM f32 port caps the stream), so
~118us; ACT exp is ~110us.  The fp8 DoubleRow matmuls + concurrent s-reduce
pull PE (~150us in the all-bf16 baseline) under that floor.
"""

import numpy as np
import ml_dtypes

BF16 = ml_dtypes.bfloat16
FP8 = ml_dtypes.float8_e4m3
N_CORES = 8
D = 256
TILE = 512  # items per tile
A_SCALE = 16.0  # pre-scale on A so fp8e4 quantization stays in normals
USE_FP8 = True  # fp8 DoubleRow matmuls for edge tiles


# ----------------------------------------------------------------------------
# Bass kernel builder (one NEFF shared by all cores)
# ----------------------------------------------------------------------------

_BASS_CACHE = {}


def _get_elup1_op():
    """Register (once) a custom fused DVE op:
    out = min(in0, relu(in1) * s0 + s1).

    With in0 = exp(y/s), in1 = y and s0 = 1/s this computes elu(y/s) + 1 in
    a single VectorE pass."""
    from concourse import dve_ops
    from concourse.dve_spec import Spec, Src0, Src1, C0, C1, relu, minn, \
        lower, _has_src1
    from concourse.dve_uop import DveOpSpec

    name = "ELUP1S_ANT"
    for o in dve_ops.OPS:
        if o.name == name:
            return o

    def ref(in0, in1, s0, s1, imm2):
        return np.minimum(
            in0.astype(np.float32),
            np.maximum(in1.astype(np.float32), 0.0) * s0 + s1,
        ).astype(np.float32)

    spec = Spec(body=minn(Src0, relu(Src1) * C0 + C1), reference=ref)
    row = dve_ops._CUSTOM_DVE_ROW_BASE + len(dve_ops.OPS)
    shas = {}
    for ver in ("v3", "v4"):
        uops = lower(spec, ver=ver)
        shas[ver] = DveOpSpec(name=name, opcode=row, uops=uops,
                              rd1_en=_has_src1(spec)).sha(ver)
    op = dve_ops.DveOp(name, spec, subdim=False, uops_sha=shas)
    dve_ops.OPS.append(op)
    dve_ops.CUSTOM_DVE_SPECS[op.name] = op.spec
    dve_ops._SUB_OPCODE_FOR_NAME[op.name] = row
    return op


def _build_bass(n_tiles, tiles_per_seg, m_pad, node_tiles=1):
    """Build the Tile/Bass program.

    n_tiles: number of 512-item tiles per core (== 16 graphs * tiles_per_seg)
    tiles_per_seg: tiles per graph segment-pair (node tiles + edge tiles)
    """
    key = (n_tiles, tiles_per_seg, m_pad, node_tiles, USE_FP8)
    if key in _BASS_CACHE:
        return _BASS_CACHE[key]

    import concourse.mybir as mybir
    import concourse.tile as tile
    from concourse import bacc

    dt = mybir.dt
    G = 16  # graphs per core
    assert n_tiles == G * tiles_per_seg
    assert m_pad == n_tiles * TILE
    assert n_tiles <= 128
    edge_tiles = tiles_per_seg - node_tiles
    edge_dt = dt.float8e4 if USE_FP8 else dt.bfloat16

    elup1 = _get_elup1_op()
    nc = bacc.Bacc("TRN2", target_bir_lowering=False)
    items_n_d = nc.dram_tensor("items_n", [128, G * node_tiles, 2, TILE],
                               dt.bfloat16, kind="ExternalInput")
    items_e_d = nc.dram_tensor("items_e", [128, G * edge_tiles, 2, TILE],
                               edge_dt, kind="ExternalInput")
    # mats_*[p, g, dc, kc, m] = A[g][kc*128+p][dc*128+m] (x A_SCALE)
    mats_n_d = nc.dram_tensor("mats_n", [128, G, 2, 2, 128], dt.bfloat16,
                              kind="ExternalInput")
    mats_e_d = nc.dram_tensor("mats_e", [128, G, 2, 2, 128], edge_dt,
                              kind="ExternalInput")
    wtab_d = nc.dram_tensor("wtab", [128, 2 * 2 * 32 * 32], dt.bfloat16,
                            kind="ExternalInput")
    s_d = nc.dram_tensor("s_out", [128, 2 * TILE], dt.float32,
                         kind="ExternalOutput")

    with tile.TileContext(nc) as tc:
        with (
            tc.tile_pool(name="const", bufs=1) as const_pool,
            tc.tile_pool(name="items", bufs=14) as item_pool,
            tc.tile_pool(name="psum_y", bufs=3, space="PSUM") as ypool,
            tc.tile_pool(name="psum_s", bufs=1, space="PSUM") as spool,
            tc.tile_pool(name="elu", bufs=4) as elu_pool,
            tc.tile_pool(name="sout", bufs=1) as sout_pool,
        ):
            # Consts: ACT HWDGE path, one pool tile per chunk so the chunk
            # DMAs carry no same-tile WAW deps (they'd serialize otherwise).
            MCH = 8  # graphs per mats chunk
            mats_n_sbs = [const_pool.tile([128, MCH, 2, 2, 128], dt.bfloat16,
                                          name=f"matnb{i}", tag=f"matn{i}")
                          for i in range(G // MCH)]
            mats_e_sbs = [const_pool.tile([128, MCH, 2, 2, 128], edge_dt,
                                          name=f"mateb{i}", tag=f"mate{i}")
                          for i in range(G // MCH)]
            wtab_sbs = [const_pool.tile([128, 8 * 128], dt.bfloat16,
                                        name=f"wtabb{i}", tag=f"wtab{i}")
                        for i in range(4)]

            def load_mats(which, ch, lo=0, hi=MCH):
                sbs, d = ((mats_n_sbs, mats_n_d) if which == 0 else
                          (mats_e_sbs, mats_e_d))
                nc.scalar.dma_start(sbs[ch][:, lo:hi, :, :, :],
                                    d[:, ch * MCH + lo:ch * MCH + hi, :, :, :])

            def load_wtab(ch):
                sl = slice(ch * 8 * 128, (ch + 1) * 8 * 128)
                nc.scalar.dma_start(wtab_sbs[ch][:], wtab_d[:, sl])

            # s accumulator: 2 PSUM banks.  Tile t (kc half h) accumulates
            # its 512 item sums at rows 64*h + (t%64), cols (t//64)*512.
            # memset first so the end-of-kernel drain never reads
            # uninitialized PSUM (row/col corners no tile maps to).
            psum_s = spool.tile([128, 2 * TILE], dt.float32)
            nc.vector.memset(psum_s[:], 0)

            # HAM pre-warm: keep PE busy during the DMA preamble so real
            # matmuls start at 2.4 GHz instead of ramping from 1.2 GHz.
            # Warm output parks in an s-accumulator corner no tile uses
            # (rows 32:64 of the cc=1 column half).
            warm_sb = const_pool.tile([128, 64], dt.bfloat16)
            nc.vector.memset(warm_sb[:], 0)
            for _ in range(56):
                nc.tensor.matmul(psum_s[32:64, TILE:TILE + 64],
                                 warm_sb[:, 0:32], warm_sb[:],
                                 start=True, stop=True, skip_group_check=True)

            # First graph's matrices and the first w chunk load as small
            # DMAs so tile 0's matmuls aren't gated on megabyte transfers;
            # the rest stream in while early tiles compute.
            load_mats(0, 0, 0, 1)
            load_mats(1, 0, 0, 1)
            load_wtab(0)
            load_mats(0, 0, 1, 2)
            load_mats(1, 0, 1, 2)
            load_wtab(1)
            const_sched = {1: lambda: (load_mats(0, 0, 2, 4),
                                       load_mats(1, 0, 2, 4)),
                           2: lambda: load_wtab(2),
                           3: lambda: (load_mats(0, 0, 4, 8),
                                       load_mats(1, 0, 4, 8)),
                           4: lambda: load_wtab(3),
                           8: lambda: load_mats(0, 1),
                           16: lambda: load_mats(1, 1)}

            def w_sl(typ, kc, c):
                # c-major so tile t only depends on wtab chunk c // 8
                ch, cc = divmod(c, 8)
                off = ((cc * 2 + typ) * 2 + kc) * 32
                return wtab_sbs[ch][:, off:off + 32]

            s_sb = sout_pool.tile([128, 2 * TILE], dt.float32)
            pending_s = []
            for t in range(n_tiles):
                gl, r = divmod(t, tiles_per_seg)
                is_node = r < node_tiles
                typ = 0 if is_node else 1
                ch, g = divmod(gl, MCH)

                if t in const_sched:
                    const_sched[t]()
                if is_node:
                    x2 = item_pool.tile([128, 2, TILE], dt.bfloat16, tag="xn")
                    ti = gl * node_tiles + r
                    nc.sync.dma_start(x2[:, :, :], items_n_d[:, ti, :, :])
                else:
                    x2 = item_pool.tile([128, 2, TILE], edge_dt, tag="xe")
                    ti = gl * edge_tiles + (r - node_tiles)
                    nc.sync.dma_start(x2[:, :, :], items_e_d[:, ti, :, :])

                # both d-chunks side by side in one 2-bank PSUM tile
                y = ypool.tile([128, 2 * TILE], dt.float32, tag="y")
                for dc in range(2):
                    ysl = y[:, dc * TILE:(dc + 1) * TILE]
                    if not is_node and USE_FP8:
                        nc.tensor.matmul(
                            ysl, mats_e_sbs[ch][:, g, dc, :, :], x2[:, :, :],
                            start=True, stop=True,
                            perf_mode=mybir.MatmulPerfMode.DoubleRow)
                    else:
                        sbs = mats_n_sbs if is_node else mats_e_sbs
                        nc.tensor.matmul(ysl, sbs[ch][:, g, dc, 0, :],
                                         x2[:, 0, :], start=True, stop=False)
                        nc.tensor.matmul(ysl, sbs[ch][:, g, dc, 1, :],
                                         x2[:, 1, :], start=False, stop=True)
                e_t = elu_pool.tile([128, 2 * TILE], dt.bfloat16, tag="e")
                nc.scalar.activation(e_t[:], y[:],
                                     mybir.ActivationFunctionType.Exp,
                                     scale=1.0 / A_SCALE)
                el_t = elu_pool.tile([128, 2 * TILE], dt.bfloat16, tag="el")
                nc.vector._custom_dve(elup1, out=el_t[:], in0=e_t[:],
                                      in1=y[:], s0=1.0 / A_SCALE, s1=1.0)

                # defer this tile's s-reduce matmuls by one tile so the
                # ACT->DVE chain has a full tile of slack before PE needs
                # el_t.  The kc halves go to different 32-col array groups
                # (concurrent) and different PSUM row halves.
                def s_mms(typ=typ, el_t=el_t, t=t):
                    cc, rr = divmod(t, 64)
                    g2, c = divmod(rr, 32)
                    csl = slice(cc * TILE, (cc + 1) * TILE)
                    for kc in range(2):
                        r0 = 64 * kc + 32 * g2
                        nc.tensor.matmul(
                            psum_s[r0:r0 + 32, csl], w_sl(typ, kc, c),
                            el_t[:, kc * TILE:(kc + 1) * TILE],
                            start=(c == 0), stop=(c == 31 or t == n_tiles - 1),
                            tile_position=(0, r0), skip_group_check=True)
                pending_s.append(s_mms)
                if len(pending_s) > 1:
                    pending_s.pop(0)()

                # the cc=0 column half is complete once tile 63's s-matmuls
                # have run (issued at t=64); drain it on ScalarE (which has
                # slack) in quarter-bank pieces overlapped with the
                # remaining tiles, so the end-of-kernel tail only covers
                # the cc=1 half
                if t in (66, 70) and n_tiles > 64:
                    half = slice(0, TILE // 2) if t == 66 else \
                        slice(TILE // 2, TILE)
                    nc.scalar.copy(out=s_sb[:, half], in_=psum_s[:, half])
                    nc.scalar.dma_start(s_d[:, half], s_sb[:, half])

            for fn in pending_s:
                fn()

            tail = slice(TILE, 2 * TILE) if n_tiles > 64 else slice(0, 2 * TILE)
            nc.scalar.copy(out=s_sb[:, tail], in_=psum_s[:, tail])
            nc.scalar.dma_start(s_d[:, tail], s_sb[:, tail])

    nc.compile()
    _BASS_CACHE[key] = nc
    return nc


# ----------------------------------------------------------------------------
# Host-side wrapper
# ----------------------------------------------------------------------------

def kernel(instruction_batch, distribution, node_prop_similarities,
           relation_similarity, node_attrs, edge_attrs,
           W_node_props, W_edge, w_node_score, w_rel_score,
           node_indices, edge_batch_indices, edge_indices):
    from concourse.bass_utils import run_bass_kernel_spmd

    ib = np.asarray(instruction_batch, dtype=np.float32)
    dist = np.asarray(distribution, dtype=np.float32)
    sim = np.asarray(node_prop_similarities, dtype=np.float32)
    rsim = np.asarray(relation_similarity, dtype=np.float32)
    na = np.asarray(node_attrs, dtype=np.float32)
    ea = np.asarray(edge_attrs, dtype=np.float32)
    Wp = np.asarray(W_node_props, dtype=np.float32)
    We = np.asarray(W_edge, dtype=np.float32)
    wn = np.asarray(w_node_score, dtype=np.float32)
    wr = np.asarray(w_rel_score, dtype=np.float32)
    ni = np.asarray(node_indices).astype(np.int64)
    ebi = np.asarray(edge_batch_indices).astype(np.int64)
    ei = np.asarray(edge_indices).astype(np.int64)
    src, dst = ei[0], ei[1]

    edge_np_dt = FP8 if USE_FP8 else BF16
    B = ib.shape[0]
    N = na.shape[0]
    G = B // N_CORES  # graphs per core

    cn = np.bincount(ni, minlength=B)
    ce = np.bincount(ebi, minlength=B)
    pad_n = max(TILE, int(-(-cn.max() // TILE)) * TILE)
    pad_e = max(TILE, int(-(-ce.max() // TILE)) * TILE)
    seg_items = pad_n + pad_e
    tiles_per_seg = seg_items // TILE
    node_tiles = pad_n // TILE
    edge_tiles = pad_e // TILE
    n_tiles = G * tiles_per_seg
    m_pad = n_tiles * TILE
    assert n_tiles <= 128, "s accumulator overflow; shrink TILE padding"

    nstart = np.concatenate([[0], np.cumsum(cn)])
    eperm = np.argsort(ebi, kind="stable")
    estart = np.concatenate([[0], np.cumsum(ce)])

    # ---- item columns, transposed + narrowed, padded per graph ----
    na_c = na.astype(BF16)
    ea_c = ea[eperm].astype(edge_np_dt)
    # interleaved layout: [dev][128 p][tile][kc][512] so one DMA per tile
    items_n = np.zeros((N_CORES, 128, G * node_tiles, 2, TILE), dtype=BF16)
    items_e = np.zeros((N_CORES, 128, G * edge_tiles, 2, TILE),
                       dtype=edge_np_dt)

    def put(arr, dev, col0, block):
        # block: [n_items, 256] -> scatter columns col0..col0+n
        n = block.shape[0]
        bT = block.T.reshape(2, 128, n)  # [kc, p, n]
        j = np.arange(col0, col0 + n)
        tt, jj = j // TILE, j % TILE
        arr[dev][:, tt, 0, jj] = bT[0]
        arr[dev][:, tt, 1, jj] = bT[1]

    for g in range(B):
        dev, gl = divmod(g, G)
        put(items_n, dev, gl * pad_n, na_c[nstart[g]:nstart[g + 1]])
        put(items_e, dev, gl * pad_e, ea_c[estart[g]:estart[g + 1]])

    # ---- per-graph matrices A[k, d] (instr folded in), x A_SCALE ----
    C = np.einsum("gp,pde->gde", sim, Wp)
    A_node = (C * ib[:, :, None]).transpose(0, 2, 1) * A_SCALE  # [g, k, d]
    A_edge = (We[None, :, :] * ib[:, :, None]).transpose(0, 2, 1) * A_SCALE

    def mats_blob(A, np_dt):
        # A: [B, 256 k, 256 d] -> [dev, p, g, dc, kc, m]
        Ad = A.astype(np_dt).reshape(N_CORES, G, 2, 128, 2, 128)
        return np.ascontiguousarray(Ad.transpose(0, 3, 1, 4, 2, 5))

    mats_n = mats_blob(A_node, BF16)
    mats_e = mats_blob(A_edge, edge_np_dt)

    # ---- w tables: wtab[k, ((c*2+typ)*2+kc)*32+m] = w_typ[kc*128+k]*(m==c)
    wt = np.stack([wn, wr]).astype(np.float32)                  # [2, 256]
    eye = np.eye(32, dtype=np.float32)
    wtab = np.einsum("tk,cm->kctm", wt.reshape(2, 2, 128).reshape(4, 128), eye)
    wtab = np.ascontiguousarray(wtab.reshape(128, 32, 2, 2, 32)
                                ).reshape(128, 4 * 32 * 32).astype(BF16)

    # ---- run on 8 cores ----
    nc = _build_bass(n_tiles, tiles_per_seg, m_pad, node_tiles)
    in_maps = [{"items_n": items_n[d], "items_e": items_e[d],
                "mats_n": mats_n[d], "mats_e": mats_e[d], "wtab": wtab}
               for d in range(N_CORES)]
    res = run_bass_kernel_spmd(nc, in_maps, core_ids=list(range(N_CORES)))
    s_rows = np.stack([r["s_out"] for r in res.results])        # [8, 128, 1024]

    # ---- unshard + finish on host ----
    # tile t's 512 sums: rows (t%64) + (64 + t%64), cols (t//64)*512
    sum_wn = float(wt[0].astype(BF16).astype(np.float32).sum())
    sum_wr = float(wt[1].astype(BF16).astype(np.float32).sum())
    s_flat = np.empty((N_CORES, n_tiles * TILE), np.float32)
    for dev in range(N_CORES):
        S = s_rows[dev]
        for cc in range((n_tiles + 63) // 64):
            nt = min(64, n_tiles - cc * 64)
            blk = (S[0:nt, cc * TILE:(cc + 1) * TILE]
                   + S[64:64 + nt, cc * TILE:(cc + 1) * TILE])
            s_flat[dev, cc * 64 * TILE:(cc * 64 + nt) * TILE] = blk.reshape(-1)

    state_logits = np.empty(N, np.float32)
    s_e = np.empty(ei.shape[1], np.float32)
    for g in range(B):
        dev, gl = divmod(g, G)
        rows = s_flat[dev][gl * seg_items:(gl + 1) * seg_items]
        state_logits[nstart[g]:nstart[g + 1]] = rows[:cn[g]] - sum_wn
        s_e[estart[g]:estart[g + 1]] = rows[pad_n:pad_n + ce[g]] - sum_wr

    rel_logits = np.bincount(dst[eperm], weights=dist[src[eperm]] * s_e,
                             minlength=N).astype(np.float32)

    def seg_softmax(x):
        mx = np.maximum.reduceat(x, nstart[:-1])
        ex = np.exp(x - mx[ni])
        sm = np.add.reduceat(ex, nstart[:-1])
        return ex / sm[ni]

    r = rsim[ni]
    out = r * seg_softmax(rel_logits) + (1.0 - r) * seg_softmax(state_logits)
    return out.astype(np.float32)


# revision 11
# speedup vs baseline: 1.0022x; 1.0022x over previous
"""Trainium2 Bass kernel for nn_NSMCell (GNN message passing).

Strategy
--------
The reference output is only [N]: a per-graph blend of two segment softmaxes
over per-node scalars.  Both scalars are of the form

    s_i = sum_d w_d * elu( M_g[d, :] @ x_i )

where for "node items" M_g = (sim[g] . W_node_props) * instr[g] and x = node
attr, and for "edge items" M_g = W_edge * instr[g] and x = edge attr.  The
per-graph matrices are built on the host (they are tiny); the device streams
all item columns through the PE + exp/min elu + a weighted partition reduce.
Graphs (16 per core) are sharded across the 8 cores; every graph's node and
edge items are padded to fixed per-graph segment sizes so a single NEFF
serves all cores.  The edge-message scatter (index_add) collapses to a
host-side bincount of per-edge scalars, and the segment softmax + blend run
on the host over [N] values (negligible work).

Device layout per 512-item tile (d on partitions, 2 chunks of 128 side by
side in one 2-bank PSUM tile; all A matrices pre-scaled x16):
  y[d, e]   = A_seg[k, d]^T @ xT[k, e]   edge tiles: 2 fp8e4 DoubleRow
                                         matmuls (K=256 virtualized);
                                         node tiles: 4 bf16 matmuls (node
                                         y has ~4x the variance - fp8
                                         there costs ~4% logit error)
  E         = exp(y/16)                  ScalarE, PSUM -> SBUF bf16
  EL        = min(E, relu(y)/16 + 1)     one fused custom VectorE op
  s-rows   += (w (x) delta_c)^T @ EL     2 bf16 matmuls into separate PSUM
                                         col-groups (concurrent via 32-col
                                         array tiling); kc halves land in
                                         different row halves, host adds
The s accumulator is a 2-bank PSUM tile ([128, 1024]); tile t's 512 item
sums land at rows (t%64, 64+t%64), cols (t//64)*512.  Drained once at the
end; the host adds the kc half-rows and subtracts sum(w) to undo the +1.

The elementwise stage is the wall: DVE reads y from PSUM at 1 elem/lane/cyc
(custom DVE ops have no 2x mode and the PSUM f32 port caps the stream), so
~118us; ACT exp is ~110us.  The fp8 DoubleRow matmuls + concurrent s-reduce
pull PE (~150us in the all-bf16 baseline) under that floor.
"""

import numpy as np
import ml_dtypes

BF16 = ml_dtypes.bfloat16
FP8 = ml_dtypes.float8_e4m3
N_CORES = 8
D = 256
TILE = 512  # items per tile
A_SCALE = 16.0  # pre-scale on A so fp8e4 quantization stays in normals
USE_FP8 = True  # fp8 DoubleRow matmuls for edge tiles


# ----------------------------------------------------------------------------
# Bass kernel builder (one NEFF shared by all cores)
# ----------------------------------------------------------------------------

_BASS_CACHE = {}


def _get_elup1_op():
    """Register (once) a custom fused DVE op:
    out = min(in0, relu(in1) * s0 + s1).

    With in0 = exp(y/s), in1 = y and s0 = 1/s this computes elu(y/s) + 1 in
    a single VectorE pass."""
    from concourse import dve_ops
    from concourse.dve_spec import Spec, Src0, Src1, C0, C1, relu, minn, \
        lower, _has_src1
    from concourse.dve_uop import DveOpSpec

    name = "ELUP1S_ANT"
    for o in dve_ops.OPS:
        if o.name == name:
            return o

    def ref(in0, in1, s0, s1, imm2):
        return np.minimum(
            in0.astype(np.float32),
            np.maximum(in1.astype(np.float32), 0.0) * s0 + s1,
        ).astype(np.float32)

    spec = Spec(body=minn(Src0, relu(Src1) * C0 + C1), reference=ref)
    row = dve_ops._CUSTOM_DVE_ROW_BASE + len(dve_ops.OPS)
    shas = {}
    for ver in ("v3", "v4"):
        uops = lower(spec, ver=ver)
        shas[ver] = DveOpSpec(name=name, opcode=row, uops=uops,
                              rd1_en=_has_src1(spec)).sha(ver)
    op = dve_ops.DveOp(name, spec, subdim=False, uops_sha=shas)
    dve_ops.OPS.append(op)
    dve_ops.CUSTOM_DVE_SPECS[op.name] = op.spec
    dve_ops._SUB_OPCODE_FOR_NAME[op.name] = row
    return op


def _build_bass(n_tiles, tiles_per_seg, m_pad, node_tiles=1):
    """Build the Tile/Bass program.

    n_tiles: number of 512-item tiles per core (== 16 graphs * tiles_per_seg)
    tiles_per_seg: tiles per graph segment-pair (node tiles + edge tiles)
    """
    key = (n_tiles, tiles_per_seg, m_pad, node_tiles, USE_FP8)
    if key in _BASS_CACHE:
        return _BASS_CACHE[key]

    import concourse.mybir as mybir
    import concourse.tile as tile
    from concourse import bacc

    dt = mybir.dt
    G = 16  # graphs per core
    assert n_tiles == G * tiles_per_seg
    assert m_pad == n_tiles * TILE
    assert n_tiles <= 128
    edge_tiles = tiles_per_seg - node_tiles
    edge_dt = dt.float8e4 if USE_FP8 else dt.bfloat16

    elup1 = _get_elup1_op()
    nc = bacc.Bacc("TRN2", target_bir_lowering=False)
    items_n_d = nc.dram_tensor("items_n", [128, G * node_tiles, 2, TILE],
                               dt.bfloat16, kind="ExternalInput")
    items_e_d = nc.dram_tensor("items_e", [128, G * edge_tiles, 2, TILE],
                               edge_dt, kind="ExternalInput")
    # mats_*[p, g, dc, kc, m] = A[g][kc*128+p][dc*128+m] (x A_SCALE)
    mats_n_d = nc.dram_tensor("mats_n", [128, G, 2, 2, 128], dt.bfloat16,
                              kind="ExternalInput")
    mats_e_d = nc.dram_tensor("mats_e", [128, G, 2, 2, 128], edge_dt,
                              kind="ExternalInput")
    wtab_d = nc.dram_tensor("wtab", [128, 2 * 2 * 32 * 32], dt.bfloat16,
                            kind="ExternalInput")
    s_d = nc.dram_tensor("s_out", [128, 2 * TILE], dt.float32,
                         kind="ExternalOutput")

    with tile.TileContext(nc) as tc:
        with (
            tc.tile_pool(name="const", bufs=1) as const_pool,
            tc.tile_pool(name="items", bufs=14) as item_pool,
            tc.tile_pool(name="psum_y", bufs=3, space="PSUM") as ypool,
            tc.tile_pool(name="psum_s", bufs=1, space="PSUM") as spool,
            tc.tile_pool(name="elu", bufs=4) as elu_pool,
            tc.tile_pool(name="sout", bufs=1) as sout_pool,
        ):
            # Consts: ACT HWDGE path, one pool tile per chunk so the chunk
            # DMAs carry no same-tile WAW deps (they'd serialize otherwise).
            MCH = 8  # graphs per mats chunk
            mats_n_sbs = [const_pool.tile([128, MCH, 2, 2, 128], dt.bfloat16,
                                          name=f"matnb{i}", tag=f"matn{i}")
                          for i in range(G // MCH)]
            mats_e_sbs = [const_pool.tile([128, MCH, 2, 2, 128], edge_dt,
                                          name=f"mateb{i}", tag=f"mate{i}")
                          for i in range(G // MCH)]
            wtab_sbs = [const_pool.tile([128, 8 * 128], dt.bfloat16,
                                        name=f"wtabb{i}", tag=f"wtab{i}")
                        for i in range(4)]

            def load_mats(which, ch, lo=0, hi=MCH):
                sbs, d = ((mats_n_sbs, mats_n_d) if which == 0 else
                          (mats_e_sbs, mats_e_d))
                nc.scalar.dma_start(sbs[ch][:, lo:hi, :, :, :],
                                    d[:, ch * MCH + lo:ch * MCH + hi, :, :, :])

            def load_wtab(ch):
                sl = slice(ch * 8 * 128, (ch + 1) * 8 * 128)
                nc.scalar.dma_start(wtab_sbs[ch][:], wtab_d[:, sl])

            # s accumulator: 2 PSUM banks.  Tile t (kc half h) accumulates
            # its 512 item sums at rows 64*h + (t%64), cols (t//64)*512.
            # memset first so the end-of-kernel drain never reads
            # uninitialized PSUM (row/col corners no tile maps to).
            psum_s = spool.tile([128, 2 * TILE], dt.float32)
            nc.vector.memset(psum_s[:], 0)

            # HAM pre-warm: keep PE busy during the DMA preamble so real
            # matmuls start at 2.4 GHz instead of ramping from 1.2 GHz.
            # Warm output parks in an s-accumulator corner no tile uses
            # (rows 32:64 of the cc=1 column half).
            warm_sb = const_pool.tile([128, 64], dt.bfloat16)
            nc.vector.memset(warm_sb[:], 0)
            for _ in range(56):
                nc.tensor.matmul(psum_s[32:64, TILE:TILE + 64],
                                 warm_sb[:, 0:32], warm_sb[:],
                                 start=True, stop=True, skip_group_check=True)

            # Graph 0's matrices load first (small DMAs) so tile 0's
            # matmuls aren't gated on megabyte transfers; the rest stream
            # in while early tiles compute.  Each DMA costs 128 row
            # descriptors regardless of size, so don't split further.
            const_sched = {0: lambda: (load_mats(0, 0, 0, 1),
                                       load_mats(1, 0, 0, 1),
                                       load_wtab(0)),
                           1: lambda: (load_mats(0, 0, 1, 8),
                                       load_mats(1, 0, 1, 8),
                                       load_wtab(1)),
                           3: lambda: load_wtab(2),
                           5: lambda: load_wtab(3),
                           8: lambda: load_mats(0, 1),
                           16: lambda: load_mats(1, 1)}

            def w_sl(typ, kc, c):
                # c-major so tile t only depends on wtab chunk c // 8
                ch, cc = divmod(c, 8)
                off = ((cc * 2 + typ) * 2 + kc) * 32
                return wtab_sbs[ch][:, off:off + 32]

            s_sb = sout_pool.tile([128, 2 * TILE], dt.float32)
            pending_s = []
            for t in range(n_tiles):
                gl, r = divmod(t, tiles_per_seg)
                is_node = r < node_tiles
                typ = 0 if is_node else 1
                ch, g = divmod(gl, MCH)

                if is_node:
                    x2 = item_pool.tile([128, 2, TILE], dt.bfloat16, tag="xn")
                    ti = gl * node_tiles + r
                    nc.sync.dma_start(x2[:, :, :], items_n_d[:, ti, :, :])
                else:
                    x2 = item_pool.tile([128, 2, TILE], edge_dt, tag="xe")
                    ti = gl * edge_tiles + (r - node_tiles)
                    nc.sync.dma_start(x2[:, :, :], items_e_d[:, ti, :, :])
                if t in const_sched:
                    const_sched[t]()

                # both d-chunks side by side in one 2-bank PSUM tile
                y = ypool.tile([128, 2 * TILE], dt.float32, tag="y")
                for dc in range(2):
                    ysl = y[:, dc * TILE:(dc + 1) * TILE]
                    if not is_node and USE_FP8:
                        nc.tensor.matmul(
                            ysl, mats_e_sbs[ch][:, g, dc, :, :], x2[:, :, :],
                            start=True, stop=True,
                            perf_mode=mybir.MatmulPerfMode.DoubleRow)
                    else:
                        sbs = mats_n_sbs if is_node else mats_e_sbs
                        nc.tensor.matmul(ysl, sbs[ch][:, g, dc, 0, :],
                                         x2[:, 0, :], start=True, stop=False)
                        nc.tensor.matmul(ysl, sbs[ch][:, g, dc, 1, :],
                                         x2[:, 1, :], start=False, stop=True)
                e_t = elu_pool.tile([128, 2 * TILE], dt.bfloat16, tag="e")
                nc.scalar.activation(e_t[:], y[:],
                                     mybir.ActivationFunctionType.Exp,
                                     scale=1.0 / A_SCALE)
                el_t = elu_pool.tile([128, 2 * TILE], dt.bfloat16, tag="el")
                nc.vector._custom_dve(elup1, out=el_t[:], in0=e_t[:],
                                      in1=y[:], s0=1.0 / A_SCALE, s1=1.0)

                # defer this tile's s-reduce matmuls by one tile so the
                # ACT->DVE chain has a full tile of slack before PE needs
                # el_t.  The kc halves go to different 32-col array groups
                # (concurrent) and different PSUM row halves.
                def s_mms(typ=typ, el_t=el_t, t=t):
                    cc, rr = divmod(t, 64)
                    g2, c = divmod(rr, 32)
                    csl = slice(cc * TILE, (cc + 1) * TILE)
                    for kc in range(2):
                        r0 = 64 * kc + 32 * g2
                        nc.tensor.matmul(
                            psum_s[r0:r0 + 32, csl], w_sl(typ, kc, c),
                            el_t[:, kc * TILE:(kc + 1) * TILE],
                            start=(c == 0), stop=(c == 31 or t == n_tiles - 1),
                            tile_position=(0, r0), skip_group_check=True)
                pending_s.append(s_mms)
                if len(pending_s) > 1:
                    pending_s.pop(0)()

                # the cc=0 column half is complete once tile 63's s-matmuls
                # have run (issued at t=64); drain it on ScalarE (which has
                # slack) in quarter-bank pieces overlapped with the
                # remaining tiles, so the end-of-kernel tail only covers
                # the cc=1 half
                if t in (70, 74) and n_tiles > 64:
                    half = slice(0, TILE // 2) if t == 70 else \
                        slice(TILE // 2, TILE)
                    nc.scalar.copy(out=s_sb[:, half], in_=psum_s[:, half])
                    nc.scalar.dma_start(s_d[:, half], s_sb[:, half])

            for fn in pending_s:
                fn()

            tail = slice(TILE, 2 * TILE) if n_tiles > 64 else slice(0, 2 * TILE)
            nc.scalar.copy(out=s_sb[:, tail], in_=psum_s[:, tail])
            nc.scalar.dma_start(s_d[:, tail], s_sb[:, tail])

    nc.compile()
    _BASS_CACHE[key] = nc
    return nc


# ----------------------------------------------------------------------------
# Host-side wrapper
# ----------------------------------------------------------------------------

def kernel(instruction_batch, distribution, node_prop_similarities,
           relation_similarity, node_attrs, edge_attrs,
           W_node_props, W_edge, w_node_score, w_rel_score,
           node_indices, edge_batch_indices, edge_indices):
    from concourse.bass_utils import run_bass_kernel_spmd

    ib = np.asarray(instruction_batch, dtype=np.float32)
    dist = np.asarray(distribution, dtype=np.float32)
    sim = np.asarray(node_prop_similarities, dtype=np.float32)
    rsim = np.asarray(relation_similarity, dtype=np.float32)
    na = np.asarray(node_attrs, dtype=np.float32)
    ea = np.asarray(edge_attrs, dtype=np.float32)
    Wp = np.asarray(W_node_props, dtype=np.float32)
    We = np.asarray(W_edge, dtype=np.float32)
    wn = np.asarray(w_node_score, dtype=np.float32)
    wr = np.asarray(w_rel_score, dtype=np.float32)
    ni = np.asarray(node_indices).astype(np.int64)
    ebi = np.asarray(edge_batch_indices).astype(np.int64)
    ei = np.asarray(edge_indices).astype(np.int64)
    src, dst = ei[0], ei[1]

    edge_np_dt = FP8 if USE_FP8 else BF16
    B = ib.shape[0]
    N = na.shape[0]
    G = B // N_CORES  # graphs per core

    cn = np.bincount(ni, minlength=B)
    ce = np.bincount(ebi, minlength=B)
    pad_n = max(TILE, int(-(-cn.max() // TILE)) * TILE)
    pad_e = max(TILE, int(-(-ce.max() // TILE)) * TILE)
    seg_items = pad_n + pad_e
    tiles_per_seg = seg_items // TILE
    node_tiles = pad_n // TILE
    edge_tiles = pad_e // TILE
    n_tiles = G * tiles_per_seg
    m_pad = n_tiles * TILE
    assert n_tiles <= 128, "s accumulator overflow; shrink TILE padding"

    nstart = np.concatenate([[0], np.cumsum(cn)])
    eperm = np.argsort(ebi, kind="stable")
    estart = np.concatenate([[0], np.cumsum(ce)])

    # ---- item columns, transposed + narrowed, padded per graph ----
    na_c = na.astype(BF16)
    ea_c = ea[eperm].astype(edge_np_dt)
    # interleaved layout: [dev][128 p][tile][kc][512] so one DMA per tile
    items_n = np.zeros((N_CORES, 128, G * node_tiles, 2, TILE), dtype=BF16)
    items_e = np.zeros((N_CORES, 128, G * edge_tiles, 2, TILE),
                       dtype=edge_np_dt)

    def put(arr, dev, col0, block):
        # block: [n_items, 256] -> scatter columns col0..col0+n
        n = block.shape[0]
        bT = block.T.reshape(2, 128, n)  # [kc, p, n]
        j = np.arange(col0, col0 + n)
        tt, jj = j // TILE, j % TILE
        arr[dev][:, tt, 0, jj] = bT[0]
        arr[dev][:, tt, 1, jj] = bT[1]

    for g in range(B):
        dev, gl = divmod(g, G)
        put(items_n, dev, gl * pad_n, na_c[nstart[g]:nstart[g + 1]])
        put(items_e, dev, gl * pad_e, ea_c[estart[g]:estart[g + 1]])

    # ---- per-graph matrices A[k, d] (instr folded in), x A_SCALE ----
    C = np.einsum("gp,pde->gde", sim, Wp)
    A_node = (C * ib[:, :, None]).transpose(0, 2, 1) * A_SCALE  # [g, k, d]
    A_edge = (We[None, :, :] * ib[:, :, None]).transpose(0, 2, 1) * A_SCALE

    def mats_blob(A, np_dt):
        # A: [B, 256 k, 256 d] -> [dev, p, g, dc, kc, m]
        Ad = A.astype(np_dt).reshape(N_CORES, G, 2, 128, 2, 128)
        return np.ascontiguousarray(Ad.transpose(0, 3, 1, 4, 2, 5))

    mats_n = mats_blob(A_node, BF16)
    mats_e = mats_blob(A_edge, edge_np_dt)

    # ---- w tables: wtab[k, ((c*2+typ)*2+kc)*32+m] = w_typ[kc*128+k]*(m==c)
    wt = np.stack([wn, wr]).astype(np.float32)                  # [2, 256]
    eye = np.eye(32, dtype=np.float32)
    wtab = np.einsum("tk,cm->kctm", wt.reshape(2, 2, 128).reshape(4, 128), eye)
    wtab = np.ascontiguousarray(wtab.reshape(128, 32, 2, 2, 32)
                                ).reshape(128, 4 * 32 * 32).astype(BF16)

    # ---- run on 8 cores ----
    nc = _build_bass(n_tiles, tiles_per_seg, m_pad, node_tiles)
    in_maps = [{"items_n": items_n[d], "items_e": items_e[d],
                "mats_n": mats_n[d], "mats_e": mats_e[d], "wtab": wtab}
               for d in range(N_CORES)]
    res = run_bass_kernel_spmd(nc, in_maps, core_ids=list(range(N_CORES)))
    s_rows = np.stack([r["s_out"] for r in res.results])        # [8, 128, 1024]

    # ---- unshard + finish on host ----
    # tile t's 512 sums: rows (t%64) + (64 + t%64), cols (t//64)*512
    sum_wn = float(wt[0].astype(BF16).astype(np.float32).sum())
    sum_wr = float(wt[1].astype(BF16).astype(np.float32).sum())
    s_flat = np.empty((N_CORES, n_tiles * TILE), np.float32)
    for dev in range(N_CORES):
        S = s_rows[dev]
        for cc in range((n_tiles + 63) // 64):
            nt = min(64, n_tiles - cc * 64)
            blk = (S[0:nt, cc * TILE:(cc + 1) * TILE]
                   + S[64:64 + nt, cc * TILE:(cc + 1) * TILE])
            s_flat[dev, cc * 64 * TILE:(cc * 64 + nt) * TILE] = blk.reshape(-1)

    state_logits = np.empty(N, np.float32)
    s_e = np.empty(ei.shape[1], np.float32)
    for g in range(B):
        dev, gl = divmod(g, G)
        rows = s_flat[dev][gl * seg_items:(gl + 1) * seg_items]
        state_logits[nstart[g]:nstart[g + 1]] = rows[:cn[g]] - sum_wn
        s_e[estart[g]:estart[g + 1]] = rows[pad_n:pad_n + ce[g]] - sum_wr

    rel_logits = np.bincount(dst[eperm], weights=dist[src[eperm]] * s_e,
                             minlength=N).astype(np.float32)

    def seg_softmax(x):
        mx = np.maximum.reduceat(x, nstart[:-1])
        ex = np.exp(x - mx[ni])
        sm = np.add.reduceat(ex, nstart[:-1])
        return ex / sm[ni]

    r = rsim[ni]
    out = r * seg_softmax(rel_logits) + (1.0 - r) * seg_softmax(state_logits)
    return out.astype(np.float32)


# revision 17
# speedup vs baseline: 1.1154x; 1.1130x over previous
"""Trainium2 Bass kernel for nn_NSMCell (GNN message passing).

Strategy
--------
The reference output is only [N]: a per-graph blend of two segment softmaxes
over per-node scalars.  Both scalars are of the form

    s_i = sum_d w_d * elu( M_g[d, :] @ x_i )

where for "node items" M_g = (sim[g] . W_node_props) * instr[g] and x = node
attr, and for "edge items" M_g = W_edge * instr[g] and x = edge attr.  The
per-graph matrices are built on the host (they are tiny); the device streams
all item columns through the PE + exp/min elu + a weighted partition reduce.
Graphs (16 per core) are sharded across the 8 cores; every graph's node and
edge items are padded to fixed per-graph segment sizes so a single NEFF
serves all cores.  The edge-message scatter (index_add) collapses to a
host-side bincount of per-edge scalars, and the segment softmax + blend run
on the host over [N] values (negligible work).

Device layout per 512-item tile (d on partitions, 2 chunks of 128 side by
side in one 2-bank PSUM tile; all A matrices pre-scaled x16):
  y[d, e]   = A_seg[k, d]^T @ xT[k, e]   edge tiles: 2 fp8e4 DoubleRow
                                         matmuls (K=256 virtualized);
                                         node tiles: 4 bf16 matmuls (node
                                         y has ~4x the variance - fp8
                                         there costs ~4% logit error)
  E         = exp(y/16)                  ScalarE, PSUM -> SBUF bf16
  EL        = min(E, relu(y)/16 + 1)     one fused custom VectorE op
  s-rows   += (w (x) delta_c)^T @ EL     2 bf16 matmuls into separate PSUM
                                         col-groups (concurrent via 32-col
                                         array tiling); kc halves land in
                                         different row halves, host adds
The s accumulator is a 2-bank PSUM tile ([128, 1024]); tile t's 512 item
sums land at rows (t%64, 64+t%64), cols (t//64)*512.  Drained once at the
end; the host adds the kc half-rows and subtracts sum(w) to undo the +1.

The elementwise stage is the wall: DVE reads y from PSUM at 1 elem/lane/cyc
(custom DVE ops have no 2x mode and the PSUM f32 port caps the stream), so
~118us; ACT exp is ~110us.  The fp8 DoubleRow matmuls + concurrent s-reduce
pull PE (~150us in the all-bf16 baseline) under that floor.
"""

import numpy as np
import ml_dtypes

BF16 = ml_dtypes.bfloat16
FP8 = ml_dtypes.float8_e4m3
N_CORES = 8
D = 256
TILE = 512  # items per tile
A_SCALE = 16.0  # pre-scale on A so fp8e4 quantization stays in normals
USE_FP8 = True  # fp8 DoubleRow matmuls for edge tiles


# ----------------------------------------------------------------------------
# Bass kernel builder (one NEFF shared by all cores)
# ----------------------------------------------------------------------------

_BASS_CACHE = {}


def _get_elup1_op():
    """Register (once) a custom fused DVE op:
    out = min(in0, relu(in1) * s0 + s1).

    With in0 = exp(y/s), in1 = y and s0 = 1/s this computes elu(y/s) + 1 in
    a single VectorE pass."""
    from concourse import dve_ops
    from concourse.dve_spec import Spec, Src0, Src1, C0, C1, relu, minn, \
        lower, _has_src1
    from concourse.dve_uop import DveOpSpec

    name = "ELUP1S_ANT"
    for o in dve_ops.OPS:
        if o.name == name:
            return o

    def ref(in0, in1, s0, s1, imm2):
        return np.minimum(
            in0.astype(np.float32),
            np.maximum(in1.astype(np.float32), 0.0) * s0 + s1,
        ).astype(np.float32)

    spec = Spec(body=minn(Src0, relu(Src1) * C0 + C1), reference=ref)
    row = dve_ops._CUSTOM_DVE_ROW_BASE + len(dve_ops.OPS)
    shas = {}
    for ver in ("v3", "v4"):
        uops = lower(spec, ver=ver)
        shas[ver] = DveOpSpec(name=name, opcode=row, uops=uops,
                              rd1_en=_has_src1(spec)).sha(ver)
    op = dve_ops.DveOp(name, spec, subdim=False, uops_sha=shas)
    dve_ops.OPS.append(op)
    dve_ops.CUSTOM_DVE_SPECS[op.name] = op.spec
    dve_ops._SUB_OPCODE_FOR_NAME[op.name] = row
    return op


def _build_bass(n_tiles, tiles_per_seg, m_pad, node_tiles=1):
    """Build the Tile/Bass program.

    n_tiles: number of 512-item tiles per core (== 16 graphs * tiles_per_seg)
    tiles_per_seg: tiles per graph segment-pair (node tiles + edge tiles)
    """
    key = (n_tiles, tiles_per_seg, m_pad, node_tiles, USE_FP8)
    if key in _BASS_CACHE:
        return _BASS_CACHE[key]

    import concourse.mybir as mybir
    import concourse.tile as tile
    from concourse import bacc

    dt = mybir.dt
    G = 16  # graphs per core
    assert n_tiles == G * tiles_per_seg
    assert m_pad == n_tiles * TILE
    assert n_tiles <= 128
    edge_tiles = tiles_per_seg - node_tiles
    edge_dt = dt.float8e4 if USE_FP8 else dt.bfloat16

    def quad_of(n):
        # items DMA group size: fewer, fatter descriptors (a DMA always
        # costs 128 row descriptors, so 4 tiles per DMA = 4x bandwidth)
        for q in (4, 2, 1):
            if n % q == 0:
                return q

    qn = quad_of(G * node_tiles)
    qe = quad_of(G * edge_tiles)

    elup1 = _get_elup1_op()
    nc = bacc.Bacc("TRN2", target_bir_lowering=False)
    items_n_d = nc.dram_tensor("items_n",
                               [128, G * node_tiles // qn, qn, 2, TILE],
                               dt.bfloat16, kind="ExternalInput")
    items_e_d = nc.dram_tensor("items_e",
                               [128, G * edge_tiles // qe, qe, 2, TILE],
                               edge_dt, kind="ExternalInput")
    # mats_*[p, g, dc, kc, m] = A[g][kc*128+p][dc*128+m] (x A_SCALE)
    mats_n_d = nc.dram_tensor("mats_n", [128, G, 2, 2, 128], dt.bfloat16,
                              kind="ExternalInput")
    mats_e_d = nc.dram_tensor("mats_e", [128, G, 2, 2, 128], edge_dt,
                              kind="ExternalInput")
    wtab_d = nc.dram_tensor("wtab", [128, 2 * 2 * 32 * 32], dt.bfloat16,
                            kind="ExternalInput")
    s_d = nc.dram_tensor("s_out", [128, 2 * TILE], dt.float32,
                         kind="ExternalOutput")

    with tile.TileContext(nc) as tc:
        with (
            tc.tile_pool(name="const", bufs=1) as const_pool,
            tc.tile_pool(name="items", bufs=5) as item_pool,
            tc.tile_pool(name="psum_y", bufs=3, space="PSUM") as ypool,
            tc.tile_pool(name="psum_s", bufs=1, space="PSUM") as spool,
            tc.tile_pool(name="elu", bufs=4) as elu_pool,
            tc.tile_pool(name="sout", bufs=1) as sout_pool,
        ):
            # Consts: ACT HWDGE path, one pool tile per chunk so the chunk
            # DMAs carry no same-tile WAW deps (they'd serialize otherwise).
            MCH = 8  # graphs per mats chunk
            mats_n_sbs = [const_pool.tile([128, MCH, 2, 2, 128], dt.bfloat16,
                                          name=f"matnb{i}", tag=f"matn{i}")
                          for i in range(G // MCH)]
            mats_e_sbs = [const_pool.tile([128, MCH, 2, 2, 128], edge_dt,
                                          name=f"mateb{i}", tag=f"mate{i}")
                          for i in range(G // MCH)]
            wtab_sbs = [const_pool.tile([128, 8 * 128], dt.bfloat16,
                                        name=f"wtabb{i}", tag=f"wtab{i}")
                        for i in range(4)]

            def load_mats(which, ch, lo=0, hi=MCH):
                sbs, d = ((mats_n_sbs, mats_n_d) if which == 0 else
                          (mats_e_sbs, mats_e_d))
                nc.scalar.dma_start(sbs[ch][:, lo:hi, :, :, :],
                                    d[:, ch * MCH + lo:ch * MCH + hi, :, :, :])

            def load_wtab(ch):
                sl = slice(ch * 8 * 128, (ch + 1) * 8 * 128)
                nc.scalar.dma_start(wtab_sbs[ch][:], wtab_d[:, sl])

            # s accumulator: 2 PSUM banks.  Tile t (kc half h) accumulates
            # its 512 item sums at rows 64*h + (t%64), cols (t//64)*512.
            # memset first so the end-of-kernel drain never reads
            # uninitialized PSUM (row/col corners no tile maps to).
            psum_s = spool.tile([128, 2 * TILE], dt.float32)
            nc.vector.memset(psum_s[:], 0)

            # HAM pre-warm: keep PE busy during the DMA preamble so real
            # matmuls start at 2.4 GHz instead of ramping from 1.2 GHz.
            # Warm output parks in an s-accumulator corner no tile uses
            # (rows 32:64 of the cc=1 column half).
            warm_sb = const_pool.tile([128, 64], dt.bfloat16)
            nc.vector.memset(warm_sb[:], 0)
            for _ in range(56):
                nc.tensor.matmul(psum_s[32:64, TILE:TILE + 64],
                                 warm_sb[:, 0:32], warm_sb[:],
                                 start=True, stop=True, skip_group_check=True)

            # Graph 0's matrices load first (small DMAs) so tile 0's
            # matmuls aren't gated on megabyte transfers; the rest stream
            # in while early tiles compute.  Each DMA costs 128 row
            # descriptors regardless of size, so don't split further.
            const_sched = {0: lambda: (load_mats(0, 0, 0, 1),
                                       load_mats(1, 0, 0, 1),
                                       load_wtab(0)),
                           1: lambda: (load_mats(0, 0, 1, 8),
                                       load_mats(1, 0, 1, 8),
                                       load_wtab(1)),
                           3: lambda: load_wtab(2),
                           5: lambda: load_wtab(3),
                           8: lambda: load_mats(0, 1),
                           16: lambda: load_mats(1, 1)}

            def w_sl(typ, kc, c):
                # c-major so tile t only depends on wtab chunk c // 8
                ch, cc = divmod(c, 8)
                off = ((cc * 2 + typ) * 2 + kc) * 32
                return wtab_sbs[ch][:, off:off + 32]

            s_sb = sout_pool.tile([128, 2 * TILE], dt.float32)
            pending_s = []
            for t in range(n_tiles):
                gl, r = divmod(t, tiles_per_seg)
                is_node = r < node_tiles
                typ = 0 if is_node else 1
                ch, g = divmod(gl, MCH)

                if is_node:
                    ti = gl * node_tiles + r
                    pos = ti % qn
                    if pos == 0:
                        xq_n = item_pool.tile([128, qn, 2, TILE],
                                              dt.bfloat16, tag="xn")
                        nc.sync.dma_start(xq_n[:, :, :, :],
                                          items_n_d[:, ti // qn, :, :, :])
                    xq = xq_n
                else:
                    ti = gl * edge_tiles + (r - node_tiles)
                    pos = ti % qe
                    if pos == 0:
                        xq_e = item_pool.tile([128, qe, 2, TILE],
                                              edge_dt, tag="xe")
                        nc.sync.dma_start(xq_e[:, :, :, :],
                                          items_e_d[:, ti // qe, :, :, :])
                    xq = xq_e
                if t in const_sched:
                    const_sched[t]()

                # both d-chunks side by side in one 2-bank PSUM tile
                y = ypool.tile([128, 2 * TILE], dt.float32, tag="y")
                for dc in range(2):
                    ysl = y[:, dc * TILE:(dc + 1) * TILE]
                    if not is_node and USE_FP8:
                        nc.tensor.matmul(
                            ysl, mats_e_sbs[ch][:, g, dc, :, :],
                            xq[:, pos, :, :],
                            start=True, stop=True,
                            perf_mode=mybir.MatmulPerfMode.DoubleRow)
                    else:
                        sbs = mats_n_sbs if is_node else mats_e_sbs
                        nc.tensor.matmul(ysl, sbs[ch][:, g, dc, 0, :],
                                         xq[:, pos, 0, :],
                                         start=True, stop=False)
                        nc.tensor.matmul(ysl, sbs[ch][:, g, dc, 1, :],
                                         xq[:, pos, 1, :],
                                         start=False, stop=True)
                e_t = elu_pool.tile([128, 2 * TILE], dt.bfloat16, tag="e")
                nc.scalar.activation(e_t[:], y[:],
                                     mybir.ActivationFunctionType.Exp,
                                     scale=1.0 / A_SCALE)
                el_t = elu_pool.tile([128, 2 * TILE], dt.bfloat16, tag="el")
                nc.vector._custom_dve(elup1, out=el_t[:], in0=e_t[:],
                                      in1=y[:], s0=1.0 / A_SCALE, s1=1.0)

                # defer this tile's s-reduce matmuls by one tile so the
                # ACT->DVE chain has a full tile of slack before PE needs
                # el_t.  The kc halves go to different 32-col array groups
                # (concurrent) and different PSUM row halves.
                def s_mms(typ=typ, el_t=el_t, t=t):
                    cc, rr = divmod(t, 64)
                    g2, c = divmod(rr, 32)
                    csl = slice(cc * TILE, (cc + 1) * TILE)
                    for kc in range(2):
                        r0 = 64 * kc + 32 * g2
                        nc.tensor.matmul(
                            psum_s[r0:r0 + 32, csl], w_sl(typ, kc, c),
                            el_t[:, kc * TILE:(kc + 1) * TILE],
                            start=(c == 0), stop=(c == 31 or t == n_tiles - 1),
                            tile_position=(0, r0), skip_group_check=True)
                pending_s.append(s_mms)
                if len(pending_s) > 1:
                    pending_s.pop(0)()

                # the cc=0 column half is complete once tile 63's s-matmuls
                # have run (issued at t=64); drain it on ScalarE (which has
                # slack) in quarter-bank pieces overlapped with the
                # remaining tiles, so the end-of-kernel tail only covers
                # the cc=1 half
                if t in (70, 74) and n_tiles > 64:
                    half = slice(0, TILE // 2) if t == 70 else \
                        slice(TILE // 2, TILE)
                    nc.scalar.copy(out=s_sb[:, half], in_=psum_s[:, half])
                    nc.scalar.dma_start(s_d[:, half], s_sb[:, half])

            for fn in pending_s:
                fn()

            tail = slice(TILE, 2 * TILE) if n_tiles > 64 else slice(0, 2 * TILE)
            nc.scalar.copy(out=s_sb[:, tail], in_=psum_s[:, tail])
            nc.scalar.dma_start(s_d[:, tail], s_sb[:, tail])

    nc.compile()
    _BASS_CACHE[key] = nc
    return nc


# ----------------------------------------------------------------------------
# Host-side wrapper
# ----------------------------------------------------------------------------

def kernel(instruction_batch, distribution, node_prop_similarities,
           relation_similarity, node_attrs, edge_attrs,
           W_node_props, W_edge, w_node_score, w_rel_score,
           node_indices, edge_batch_indices, edge_indices):
    from concourse.bass_utils import run_bass_kernel_spmd

    ib = np.asarray(instruction_batch, dtype=np.float32)
    dist = np.asarray(distribution, dtype=np.float32)
    sim = np.asarray(node_prop_similarities, dtype=np.float32)
    rsim = np.asarray(relation_similarity, dtype=np.float32)
    na = np.asarray(node_attrs, dtype=np.float32)
    ea = np.asarray(edge_attrs, dtype=np.float32)
    Wp = np.asarray(W_node_props, dtype=np.float32)
    We = np.asarray(W_edge, dtype=np.float32)
    wn = np.asarray(w_node_score, dtype=np.float32)
    wr = np.asarray(w_rel_score, dtype=np.float32)
    ni = np.asarray(node_indices).astype(np.int64)
    ebi = np.asarray(edge_batch_indices).astype(np.int64)
    ei = np.asarray(edge_indices).astype(np.int64)
    src, dst = ei[0], ei[1]

    edge_np_dt = FP8 if USE_FP8 else BF16
    B = ib.shape[0]
    N = na.shape[0]
    G = B // N_CORES  # graphs per core

    cn = np.bincount(ni, minlength=B)
    ce = np.bincount(ebi, minlength=B)
    pad_n = max(TILE, int(-(-cn.max() // TILE)) * TILE)
    pad_e = max(TILE, int(-(-ce.max() // TILE)) * TILE)
    seg_items = pad_n + pad_e
    tiles_per_seg = seg_items // TILE
    node_tiles = pad_n // TILE
    edge_tiles = pad_e // TILE
    n_tiles = G * tiles_per_seg
    m_pad = n_tiles * TILE
    assert n_tiles <= 128, "s accumulator overflow; shrink TILE padding"

    nstart = np.concatenate([[0], np.cumsum(cn)])
    eperm = np.argsort(ebi, kind="stable")
    estart = np.concatenate([[0], np.cumsum(ce)])

    # ---- item columns, transposed + narrowed, padded per graph ----
    na_c = na.astype(BF16)
    ea_c = ea[eperm].astype(edge_np_dt)
    # interleaved layout: [dev][128 p][tile][kc][512] so one DMA per tile
    items_n = np.zeros((N_CORES, 128, G * node_tiles, 2, TILE), dtype=BF16)
    items_e = np.zeros((N_CORES, 128, G * edge_tiles, 2, TILE),
                       dtype=edge_np_dt)

    def put(arr, dev, col0, block):
        # block: [n_items, 256] -> scatter columns col0..col0+n
        n = block.shape[0]
        bT = block.T.reshape(2, 128, n)  # [kc, p, n]
        j = np.arange(col0, col0 + n)
        tt, jj = j // TILE, j % TILE
        arr[dev][:, tt, 0, jj] = bT[0]
        arr[dev][:, tt, 1, jj] = bT[1]

    for g in range(B):
        dev, gl = divmod(g, G)
        put(items_n, dev, gl * pad_n, na_c[nstart[g]:nstart[g + 1]])
        put(items_e, dev, gl * pad_e, ea_c[estart[g]:estart[g + 1]])

    # ---- per-graph matrices A[k, d] (instr folded in), x A_SCALE ----
    C = np.einsum("gp,pde->gde", sim, Wp)
    A_node = (C * ib[:, :, None]).transpose(0, 2, 1) * A_SCALE  # [g, k, d]
    A_edge = (We[None, :, :] * ib[:, :, None]).transpose(0, 2, 1) * A_SCALE

    def mats_blob(A, np_dt):
        # A: [B, 256 k, 256 d] -> [dev, p, g, dc, kc, m]
        Ad = A.astype(np_dt).reshape(N_CORES, G, 2, 128, 2, 128)
        return np.ascontiguousarray(Ad.transpose(0, 3, 1, 4, 2, 5))

    mats_n = mats_blob(A_node, BF16)
    mats_e = mats_blob(A_edge, edge_np_dt)

    # ---- w tables: wtab[k, ((c*2+typ)*2+kc)*32+m] = w_typ[kc*128+k]*(m==c)
    wt = np.stack([wn, wr]).astype(np.float32)                  # [2, 256]
    eye = np.eye(32, dtype=np.float32)
    wtab = np.einsum("tk,cm->kctm", wt.reshape(2, 2, 128).reshape(4, 128), eye)
    wtab = np.ascontiguousarray(wtab.reshape(128, 32, 2, 2, 32)
                                ).reshape(128, 4 * 32 * 32).astype(BF16)

    # ---- run on 8 cores ----
    nc = _build_bass(n_tiles, tiles_per_seg, m_pad, node_tiles)

    def quad_of(n):
        for q in (4, 2, 1):
            if n % q == 0:
                return q

    qn, qe = quad_of(G * node_tiles), quad_of(G * edge_tiles)
    items_n = items_n.reshape(N_CORES, 128, G * node_tiles // qn, qn, 2, TILE)
    items_e = items_e.reshape(N_CORES, 128, G * edge_tiles // qe, qe, 2, TILE)
    in_maps = [{"items_n": items_n[d], "items_e": items_e[d],
                "mats_n": mats_n[d], "mats_e": mats_e[d], "wtab": wtab}
               for d in range(N_CORES)]
    res = run_bass_kernel_spmd(nc, in_maps, core_ids=list(range(N_CORES)))
    s_rows = np.stack([r["s_out"] for r in res.results])        # [8, 128, 1024]

    # ---- unshard + finish on host ----
    # tile t's 512 sums: rows (t%64) + (64 + t%64), cols (t//64)*512
    sum_wn = float(wt[0].astype(BF16).astype(np.float32).sum())
    sum_wr = float(wt[1].astype(BF16).astype(np.float32).sum())
    s_flat = np.empty((N_CORES, n_tiles * TILE), np.float32)
    for dev in range(N_CORES):
        S = s_rows[dev]
        for cc in range((n_tiles + 63) // 64):
            nt = min(64, n_tiles - cc * 64)
            blk = (S[0:nt, cc * TILE:(cc + 1) * TILE]
                   + S[64:64 + nt, cc * TILE:(cc + 1) * TILE])
            s_flat[dev, cc * 64 * TILE:(cc * 64 + nt) * TILE] = blk.reshape(-1)

    state_logits = np.empty(N, np.float32)
    s_e = np.empty(ei.shape[1], np.float32)
    for g in range(B):
        dev, gl = divmod(g, G)
        rows = s_flat[dev][gl * seg_items:(gl + 1) * seg_items]
        state_logits[nstart[g]:nstart[g + 1]] = rows[:cn[g]] - sum_wn
        s_e[estart[g]:estart[g + 1]] = rows[pad_n:pad_n + ce[g]] - sum_wr

    rel_logits = np.bincount(dst[eperm], weights=dist[src[eperm]] * s_e,
                             minlength=N).astype(np.float32)

    def seg_softmax(x):
        mx = np.maximum.reduceat(x, nstart[:-1])
        ex = np.exp(x - mx[ni])
        sm = np.add.reduceat(ex, nstart[:-1])
        return ex / sm[ni]

    r = rsim[ni]
    out = r * seg_softmax(rel_logits) + (1.0 - r) * seg_softmax(state_logits)
    return out.astype(np.float32)
